# revision 31
# baseline (speedup 1.0000x reference)
"""Trainium2 Bass kernel for PolymorphicDenseBase (moe_routing).

out[b,u] = (1/M) * [ sum_m sim[b,m] * (x[b,:] @ kernels[m]) + sim @ biases ]
sim = softmax_m( sens_m / (1 + dist(key_b, keys_map_m)) )

Sharding: batch B=16384 split across 8 cores (2048 rows each); params
(kernels/keys_map/biases) replicated.  The 1/M factor is folded into
kcat/biases on the host.

Fully-fused per-128-row-b-tile pipeline:
  routing:  dist^2 via one augmented GEMM (norms folded into operands) ->
            dist = exp(0.5*ln(d2)) on ACT -> 1/(1+dist) via DVE approx
            reciprocal -> *sens (GpSimd) -> exp with fused rowsum on ACT ->
            simn = e/rowsum.
  bias:     PE-transpose of simn + K=32 GEMM simn^T.T @ biases -> acc init.
  main:     weight-stationary GEMM  Y[128,4096] = xT_tile.T @ Kcat  in PSUM
            (float32r: 1 cycle/row at N=512 vs 4 for fp32, ~2e-4 rel err).
            The per-mode weighted combine acc += simn[:,m]*Y_m is split
            across three engines per PSUM quarter (see ACT_MODES_PER_Q /
            GPS_MODES_PER_Q): one mode via ACT scaled-eviction (Identity
            with a [P,1] scale AP, read straight from PSUM) summed on
            GpSimd; ~one via GpSimd scale+add from the wide-evicted SBUF
            copy; the rest via DVE fused affine_then_add into two
            alternating accumulators (no long RAW chain), with the
            accumulator initialized by the first affine reading the bias
            GEMM result directly from PSUM.  GpSimd merges, DMA out.

All ACT functions used (Ln/Exp/Copy/Identity) live in one LUT table set
(see _patched_act_tables) so the activation table loads exactly once.
All DMA traffic uses pre-tiled contiguous host layouts.
"""

import functools
from contextlib import ExitStack

import numpy as np

import concourse.bacc as bacc
import concourse.hw_specs as hw_specs
import concourse.tile as tile
from concourse import mybir
from concourse import bass_utils
from concourse._compat import with_exitstack

# The act-table-load pass assigns each activation the first LUT set containing
# its function, which thrashes reloads between Ln/Exp/Copy sets.  Every ACT
# function this kernel uses (Ln, Exp, Copy, Identity) lives in the single
# "natural_log_exp_and_others" set, so restrict the table map to make the pass
# pick that set for all of them (set ids keep their original positions).
_ACT_FNS = {"Ln", "Exp", "Copy", "Identity"}
_ACT_SET = "natural_log_exp_and_others"


@functools.cache
def _patched_act_tables(module_arch):
    tabs = _patched_act_tables.__orig__(module_arch)
    out = {}
    for name, fns in tabs.items():
        if name == _ACT_SET:
            out[name] = fns
        else:
            out[name] = {f for f in fns if f.name not in _ACT_FNS}
    return out


if not getattr(hw_specs.get_activation_tables, "_moe_patched", False):
    _patched_act_tables.__orig__ = hw_specs.get_activation_tables
    _patched_act_tables._moe_patched = True
    hw_specs.get_activation_tables = _patched_act_tables
    bacc.get_activation_tables = _patched_act_tables

B, KEY, D, U, MODES = 16384, 64, 128, 128, 32
NCORES = 8
BLOC = B // NCORES            # 2048 rows per core
P = 128                       # rows per b-tile (partitions)
NT = BLOC // P                # 16 b-tiles per core
KA = KEY + 2                  # augmented key rows: [key; ||key||^2; 1]

F32 = mybir.dt.float32
# Matmul input dtype for the big GEMM. float32r streams 1 row/cycle at N>=256
# (vs 4 cycles/row for float32) with reduced-precision multiplies.
GEMM_DT = mybir.dt.float32r

MODES_PER_Q = 8               # modes per PSUM quarter-tile
NQ = MODES // MODES_PER_Q     # 4 quarter tiles per b-tile
QCOLS = MODES_PER_Q * U       # 1024 columns = 2 PSUM banks
# number of modes per quarter handled by ACT scaled-eviction + GpSimd add
ACT_MODES_PER_Q = (1, 1, 1, 1)


@with_exitstack
def _body(ctx: ExitStack, tc: tile.TileContext, keyT, xT, km, kcat, bia,
          sens_rep, eye, out):
    nc = tc.nc

    consts = ctx.enter_context(tc.tile_pool(name="consts", bufs=1))
    keyp = ctx.enter_context(tc.tile_pool(name="keyp", bufs=3))
    xp = ctx.enter_context(tc.tile_pool(name="xp", bufs=3))
    small = ctx.enter_context(tc.tile_pool(name="small", bufs=6))
    accp = ctx.enter_context(tc.tile_pool(name="accp", bufs=3))
    ysb = ctx.enter_context(tc.tile_pool(name="ysb", bufs=4))
    rps = ctx.enter_context(tc.tile_pool(name="rps", bufs=1, space="PSUM"))
    yps = ctx.enter_context(tc.tile_pool(name="yps", bufs=2, space="PSUM"))

    # ---- constants (loaded once; the big kcat load is emitted later so the
    # first b-tiles' small DMAs aren't queued behind 2MB) ----
    km_sb = consts.tile([KA, MODES], F32)
    nc.sync.dma_start(out=km_sb, in_=km[:, :])
    bia_sb = consts.tile([MODES, U], F32)
    nc.sync.dma_start(out=bia_sb, in_=bia[:, :])
    sens_sb = consts.tile([P, MODES], F32)
    nc.sync.dma_start(out=sens_sb, in_=sens_rep[:, :])
    eye_sb = consts.tile([P, P], F32)
    nc.sync.dma_start(out=eye_sb, in_=eye[:, :])
    kcat_sb = consts.tile([D, MODES * U], GEMM_DT)

    # Fully-fused per-b-tile pipeline.  Every ACT function used (Ln, Exp,
    # Copy/Identity) lives in the single natural_log_exp_and_others LUT set
    # (see the table patch above), so interleaving costs no table reloads.
    for bt in range(NT):
        # -- routing: dist = exp(0.5*ln(dist2)), dist2 via augmented GEMM --
        keyT_t = keyp.tile([KA, P], F32, tag="keyT_t")
        nc.sync.dma_start(out=keyT_t, in_=keyT[bt, :, :])
        d2 = rps.tile([P, MODES], F32, tag="d2")
        nc.tensor.matmul(d2, keyT_t, km_sb, start=True, stop=True)
        if bt == 0:
            qw = MODES * U // 4
            for kq in range(4):
                nc.sync.dma_start(out=kcat_sb[:, kq * qw:(kq + 1) * qw],
                                  in_=kcat[:, kq * qw:(kq + 1) * qw])
        lnd2 = small.tile([P, MODES], F32, tag="lnd2")
        nc.scalar.activation(out=lnd2, in_=d2,
                             func=mybir.ActivationFunctionType.Ln)
        dist = small.tile([P, MODES], F32, tag="dist")
        nc.scalar.activation(out=dist, in_=lnd2,
                             func=mybir.ActivationFunctionType.Exp, scale=0.5)

        # -- logits = sens * 1/(1+dist); e = exp(logits) --
        t1 = small.tile([P, MODES], F32, tag="t1")
        nc.scalar.activation(out=t1, in_=dist,
                             func=mybir.ActivationFunctionType.Identity,
                             bias=1.0)
        r1 = small.tile([P, MODES], F32, tag="r1")
        rscr = small.tile([P, MODES], F32, tag="rscr")
        nc.vector.reciprocal_approx_accurate(r1, t1, rscr)
        lg = small.tile([P, MODES], F32, tag="lg")
        nc.gpsimd.tensor_mul(lg, r1, sens_sb)
        # e = exp(logits); ACT's accum_out produces rowsum(e) in the same op.
        # The /MODES normalization is folded into kcat/biases on the host, so
        # simn = e / rowsum(e) here.
        e = small.tile([P, MODES], F32, tag="e")
        ssum = small.tile([P, 1], F32, tag="ssum")
        nc.scalar.activation(out=e, in_=lg,
                             func=mybir.ActivationFunctionType.Exp,
                             accum_out=ssum)
        q = small.tile([P, 1], F32, tag="q")
        nc.vector.reciprocal(q, ssum)
        simn = small.tile([P, MODES], F32, tag="simn")
        nc.vector.tensor_scalar_mul(simn, e, q[:, 0:1])

        # -- bias term: acc = (simn @ biases) via PE transpose + K=32 GEMM --
        simnT_ps = rps.tile([MODES, P], F32, tag="simnT_ps")
        nc.tensor.transpose(simnT_ps, simn, eye_sb)
        simnT_sb = small.tile([MODES, P], F32, tag="simnT_sb")
        nc.scalar.copy(simnT_sb, simnT_ps)
        bias_ps = rps.tile([P, U], F32, tag="bias_ps")
        nc.tensor.matmul(bias_ps, simnT_sb, bia_sb, start=True, stop=True)
        # two accumulators (even/odd modes) so the DVE combine isn't one long
        # RAW chain; merged after the mode loop
        acc = accp.tile([P, U], F32, tag="acc")
        nc.scalar.copy(acc, bias_ps)
        acc2 = accp.tile([P, U], F32, tag="acc2")
        nc.gpsimd.memset(acc2, 0.0)

        # -- main GEMM (stationary xT tile, streaming Kcat) + fused combine --
        # Three-engine combine split: for ACT_MODES_PER_Q modes per quarter,
        # ACT does a scaled eviction (scale = simn column) and GpSimd sums
        # those partials; DVE handles the rest with fused affine_then_add
        # from the wide-evicted SBUF copy.
        accg = accp.tile([P, U], F32, tag="accg")
        nc.gpsimd.memset(accg, 0.0)
        xT_t = xp.tile([D, P], GEMM_DT, tag="xT_t")
        nc.sync.dma_start(out=xT_t, in_=xT[bt, :, :])
        for qi in range(NQ):
            y_ps = yps.tile([P, QCOLS], F32, tag="y_ps")
            for j in range(QCOLS // 512):
                c0 = qi * QCOLS + j * 512
                nc.tensor.matmul(y_ps[:, j * 512:(j + 1) * 512], xT_t,
                                 kcat_sb[:, c0:c0 + 512],
                                 start=True, stop=True)
            ka = ACT_MODES_PER_Q[qi]
            for mj in range(ka):
                m = qi * MODES_PER_Q + mj
                tm = ysb.tile([P, U], F32, tag="tm")
                nc.scalar.activation(out=tm, in_=y_ps[:, mj * U:(mj + 1) * U],
                                     func=mybir.ActivationFunctionType.Identity,
                                     scale=simn[:, m:m + 1])
                nc.gpsimd.tensor_add(accg, accg, tm)
            # wide eviction on ACT amortizes the per-op overhead
            nwide = MODES_PER_Q - ka
            y_sb = ysb.tile([P, QCOLS], F32, tag="y_sb")
            nc.scalar.copy(y_sb[:, :nwide * U], y_ps[:, ka * U:])
            for mj in range(nwide):
                m = qi * MODES_PER_Q + ka + mj
                a = acc if (mj & 1) == 0 else acc2
                nc.vector.affine_then_add(
                    out=a,
                    in0=y_sb[:, mj * U:(mj + 1) * U],
                    in1=a,
                    scale=simn[:, m:m + 1],
                    bias=0.0,
                )
        nc.gpsimd.tensor_add(acc2, acc2, accg)
        nc.gpsimd.tensor_add(acc, acc, acc2)
        nc.sync.dma_start(out=out[bt, :, :], in_=acc)


_NC_CACHE = None


def _build_nc():
    global _NC_CACHE
    if _NC_CACHE is not None:
        return _NC_CACHE
    nc = bacc.Bacc("TRN2", target_bir_lowering=False)
    keyT = nc.dram_tensor("keyT", [NT, KA, P], F32, kind="ExternalInput")
    xT = nc.dram_tensor("xT", [NT, D, P], GEMM_DT, kind="ExternalInput")
    km = nc.dram_tensor("km", [KA, MODES], F32, kind="ExternalInput")
    kcat = nc.dram_tensor("kcat", [D, MODES * U], GEMM_DT, kind="ExternalInput")
    bia = nc.dram_tensor("bia", [MODES, U], F32, kind="ExternalInput")
    sens_rep = nc.dram_tensor("sens_rep", [P, MODES], F32, kind="ExternalInput")
    eye = nc.dram_tensor("eye", [P, P], F32, kind="ExternalInput")
    out = nc.dram_tensor("out", [NT, P, U], F32, kind="ExternalOutput")
    with tile.TileContext(nc) as tc:
        _body(tc, keyT[:], xT[:], km[:], kcat[:], bia[:], sens_rep[:],
              eye[:], out[:])
    nc.compile()
    _NC_CACHE = nc
    return nc


def _prep_host(key, x, sens, keys_map, kernels, biases):
    key = np.asarray(key, np.float32)
    x = np.asarray(x, np.float32)
    sens = np.asarray(sens, np.float32)
    keys_map = np.asarray(keys_map, np.float32)
    kernels = np.asarray(kernels, np.float32)
    biases = np.asarray(biases, np.float32)

    keyT_aug = np.empty((KA, B), np.float32)
    keyT_aug[:KEY] = key.T
    keyT_aug[KEY] = np.einsum("bk,bk->b", key, key)
    keyT_aug[KEY + 1] = 1.0

    km_aug = np.empty((KA, MODES), np.float32)
    km_aug[:KEY] = -2.0 * keys_map.T
    km_aug[KEY] = 1.0
    km_aug[KEY + 1] = np.einsum("mk,mk->m", keys_map, keys_map)

    kcat = np.ascontiguousarray(
        (kernels / MODES).transpose(1, 0, 2).reshape(D, MODES * U).astype(np.float32))
    sens_rep = np.ascontiguousarray(np.broadcast_to(sens.reshape(1, MODES),
                                                    (P, MODES)))
    eye = np.eye(P, dtype=np.float32)

    # pre-tiled contiguous layouts: [NT, rows, 128] per core
    keyT_t = np.ascontiguousarray(
        keyT_aug.reshape(KA, NCORES, NT, P).transpose(1, 2, 0, 3))
    xT_t = np.ascontiguousarray(
        x.T.reshape(D, NCORES, NT, P).transpose(1, 2, 0, 3))

    in_maps = []
    for c in range(NCORES):
        in_maps.append({
            "keyT": keyT_t[c],
            "xT": xT_t[c],
            "km": km_aug,
            "kcat": kcat,
            "bia": biases / MODES,
            "sens_rep": sens_rep,
            "eye": eye,
        })
    return in_maps


def run(inputs, trace=False, **kw):
    nc = _build_nc()
    in_maps = _prep_host(**inputs)
    res = bass_utils.run_bass_kernel_spmd(
        nc, in_maps, core_ids=list(range(NCORES)), trace=trace, **kw)
    out = np.concatenate(
        [r["out"].reshape(BLOC, U) for r in res.results], axis=0)
    return out, res


def kernel(**inputs):
    out, _ = run(inputs)
    return out
